# revision 27
# baseline (speedup 1.0000x reference)
# Trainium2 Bass kernel: BIPA MultiHeadAttention (B=32, L=577, D=768, H=12)
# Data-parallel over batch: 4 batch items per NeuronCore x 8 cores.
#
# Per-core layout (tokens padded 577 -> 640 = 5*128 per batch item;
# working width W = 580 = 2*290 chunks, one PSUM bank per chunk):
#   xT      [768, 2560]  host-transposed input, bf16
#   q,k     produced transposed:   qkT[feat, tok]  bf16 (lhsT = Wqk^T)
#   v       produced normal:       v[tok, feat]    bf16, ones col per head
#   scores  ST[j, i] = k_h^T q_h per 128-key tile; key bias (host-folded
#           alpha*mb + pad mask) is per-partition -> fused into the exp
#           activation (ScalarE) as bias, with scale = hd^-0.5.
#   AV      out^T[65, i] = [v_h | 1]^T @ exp(ST); row 64 = softmax denom.
#           PSUM banks are the scarce resource: av tiles are evacuated to
#           SBUF immediately after the AV accumulation so the next head
#           pair's matmuls don't wait on the normalize chain.
#   norm    recip (DVE) -> partition_broadcast (Pool) -> mul (DVE)
#   proj    lhsT = proj_w^T (bf16), rhs = rawT (bf16) -> +bias -> outT f32
#
# Math simplifications (exact):
#   - k bias dropped: softmax over keys is invariant to per-query shifts.
#   - v bias folded into the proj bias on host: P@(v+bv) = P@v + bv since
#     softmax rows sum to 1, and proj(out+bv) = proj(out) + proj_w@bv.
#   - alpha*mb + pad mask folded on host (alpha is a kernel input scalar).
#
# All matmuls run bf16 (1 col/cycle, same PE rate as fp32r but half the
# SBUF/DMA), accumulation in PSUM f32.

import numpy as np

B, L, D = 32, 577, 768
H, HD = 12, 64
NCORES = 8
BPC = B // NCORES            # batch items per core
LP = 640                     # padded per-batch token count (5 * 128)
NJT = LP // 128              # key/token tiles per batch item
TOK = BPC * LP               # padded tokens per core
KT = D // 128                # contraction tiles over feature dim
NQK = 12                     # q+k output feature tiles (1536 / 128)
SCALE = HD ** -0.5
NEG = -30.0                  # pad-key bias (exp(-30) ~ 9e-14)
PSTRIDE = 512                # psum chunk stride (bank aligned)

_CACHE = {}


def _build(bpc=BPC, lp=LP, lr=L, reps=1):
    # reps > 1 unrolls the whole body N times in one NEFF — used only for
    # timing (amortizes the per-dispatch host/tunnel overhead, which
    # otherwise exceeds the device time and hides it).
    import concourse.mybir as mybir
    import concourse.tile as tile
    from concourse import bacc

    f32 = mybir.dt.float32
    bf16 = mybir.dt.bfloat16
    njt = lp // 128
    tok = bpc * lp
    ch = ((lr + 1) // 2 + 1) // 2 * 2       # even half-chunk (290)
    W = 2 * ch                               # working token width (580)
    assert W <= lp
    CH = [(0, ch), (ch, W)]
    VCH = [(0, 384), (384, 768)]            # v feature chunks (6 heads each)
    NT_ORDER = [x for t in range(KT) for x in (t, KT + t)]  # q/k interleaved

    nc = bacc.Bacc(
        "TRN2",
        target_bir_lowering=False,
        debug=False,
        enable_asserts=False,
        num_devices=NCORES,
    )

    xT = nc.dram_tensor("xT", [D, tok], bf16, kind="ExternalInput").ap()
    wqkvT = nc.dram_tensor("wqkvT", [D, 3 * D], bf16, kind="ExternalInput").ap()
    pwT = nc.dram_tensor("pwT", [D, D], bf16, kind="ExternalInput").ap()
    bqkT = nc.dram_tensor("bqkT", [128, KT], f32, kind="ExternalInput").ap()
    pbT = nc.dram_tensor("pbT", [128, KT], f32, kind="ExternalInput").ap()
    mbT = nc.dram_tensor("mbT", [128, bpc * njt], f32, kind="ExternalInput").ap()
    outT = nc.dram_tensor("outT", [D, tok], f32, kind="ExternalOutput").ap()

    def sv(ap):
        # strided 2-chunk view of a psum tile: [128, 2, ch] at stride PSTRIDE
        return ap.rearrange("p (c x) -> p c x", c=2)[:, :, 0:ch]

    def cv(ap):
        # contiguous 2-chunk view of a [.., W]-wide destination
        return ap.rearrange("p (c x) -> p c x", c=2)

    with tile.TileContext(nc) as tc:
        from contextlib import ExitStack

        with ExitStack() as ctx:
            wpool = ctx.enter_context(tc.tile_pool(name="w", bufs=1))
            cpool = ctx.enter_context(tc.tile_pool(name="c", bufs=1))
            xpool = ctx.enter_context(tc.tile_pool(name="x", bufs=2))
            qkpool = ctx.enter_context(tc.tile_pool(name="qk", bufs=3))
            vpool = ctx.enter_context(tc.tile_pool(name="v", bufs=2))
            ptpool = ctx.enter_context(tc.tile_pool(name="pt", bufs=4))
            evpool = ctx.enter_context(tc.tile_pool(name="ev", bufs=2))
            rcpool = ctx.enter_context(tc.tile_pool(name="rc", bufs=2))
            stgpool = ctx.enter_context(tc.tile_pool(name="stg", bufs=2))
            rawpool = ctx.enter_context(tc.tile_pool(name="raw", bufs=2))
            opool = ctx.enter_context(tc.tile_pool(name="o", bufs=2))
            psA = ctx.enter_context(tc.tile_pool(name="psA", bufs=4, space="PSUM"))
            psB = psA

            xTr = xT.rearrange("(t p) m -> p t m", p=128)
            outTr = outT.rearrange("(t p) m -> p t m", p=128)

            # ---- resident weights / constants ----
            # Order matters for startup latency: tiny consts, then x(0), then
            # the qkv weights split q/k/v (matmuls start as slices land);
            # the proj weights are not needed until ~60us in.
            bqk = cpool.tile([128, KT], f32, tag="bqk")
            nc.sync.dma_start(bqk[:], bqkT)
            pb = cpool.tile([128, KT], f32, tag="pb")
            nc.sync.dma_start(pb[:], pbT)
            mbias = cpool.tile([128, bpc * njt], f32, tag="mbias")
            nc.sync.dma_start(mbias[:], mbT)

            xb0 = xpool.tile([128, KT, lp], bf16, tag="xb")
            nc.sync.dma_start(xb0[:], xTr[:, :, 0:lp])

            wq = wpool.tile([128, KT, 3 * D], bf16, tag="wq")
            nc.sync.dma_start(wq[:], wqkvT.rearrange("(t p) n -> p t n", p=128))
            pw = wpool.tile([128, KT, D], bf16, tag="pw")
            nc.sync.dma_start(pw[:], pwT.rearrange("(t p) n -> p t n", p=128))

            for rep in range(reps):
              for b in range(bpc):
                if b == 0 and rep == 0:
                    xb = xb0
                else:
                    xb = xpool.tile([128, KT, lp], bf16, tag="xb")
                    nc.sync.dma_start(xb[:], xTr[:, :, b * lp:(b + 1) * lp])

                # ---- q/k projection (transposed layout) ----
                qk = qkpool.tile([128, NQK, W], bf16, tag="qk")
                for nt in NT_ORDER:
                    ps = psA.tile([128, 1024], f32, tag="ps")
                    for kt in range(KT):
                        for ci, (c0, c1) in enumerate(CH):
                            nc.tensor.matmul(
                                ps[:, ci * PSTRIDE: ci * PSTRIDE + (c1 - c0)],
                                lhsT=wq[:, kt, nt * 128:(nt + 1) * 128],
                                rhs=xb[:, kt, c0:c1],
                                start=(kt == 0),
                                stop=(kt == KT - 1),
                            )
                    if nt < KT:
                        # q tile: fuse the q bias into the PSUM->SBUF move
                        nc.vector.tensor_scalar_add(
                            cv(qk[:, nt, :]), sv(ps), bqk[:, nt:nt + 1])
                    else:
                        # k tile: plain evacuation (DVE; keep Act free for exp)
                        nc.vector.tensor_copy(cv(qk[:, nt, :]), sv(ps))



                # ---- v projection (normal layout, 65-col stride per head) ----
                v = vpool.tile([128, njt, 12 * 65], bf16, tag="v")
                for mt in range(njt):
                    ps = psA.tile([128, 1024], f32, tag="ps")
                    for kt in range(KT):
                        for ci, (c0, c1) in enumerate(VCH):
                            nc.tensor.matmul(
                                ps[:, ci * PSTRIDE: ci * PSTRIDE + (c1 - c0)],
                                lhsT=xb[:, kt, mt * 128:(mt + 1) * 128],
                                rhs=wq[:, kt, 2 * D + c0: 2 * D + c1],
                                start=(kt == 0),
                                stop=(kt == KT - 1),
                            )
                    for ci, (c0, c1) in enumerate(VCH):
                        nc.vector.tensor_copy(
                            v[:, mt, ci * 6 * 65:(ci + 1) * 6 * 65].rearrange(
                                "p (h e) -> p h e", h=6)[:, :, 0:64],
                            ps[:, ci * PSTRIDE: ci * PSTRIDE + 384].rearrange(
                                "p (h e) -> p h e", h=6),
                        )
                    nc.vector.memset(
                        v[:, mt, :].rearrange("p (h e) -> p h e", h=12)[:, :, 64:65],
                        1.0,
                    )

                # ---- attention, head pairs on PE row halves ----
                raw = rawpool.tile([128, KT, W], bf16, tag="raw")
                stg = stgpool.tile([64, KT, W], bf16, tag="stg")
                for t in range(KT):
                    h0, h1 = 2 * t, 2 * t + 1
                    av0 = psB.tile([128, 1024], f32, tag="ps")
                    av1 = psB.tile([128, 1024], f32, tag="ps")
                    for jt in range(njt):
                        # truncate the last key tile to the real keys: pad
                        # keys are never read, so no masking or pad zeroing
                        # is needed anywhere (exact, not approximate).
                        j0 = jt * 128
                        kk = min((jt + 1) * 128, lr) - j0
                        st0 = psA.tile([128, 1024], f32, tag="ps")
                        st1 = psA.tile([128, 1024], f32, tag="ps")
                        for ci, (c0, c1) in enumerate(CH):
                            nc.tensor.matmul(
                                st0[0:kk, ci * PSTRIDE: ci * PSTRIDE + (c1 - c0)],
                                lhsT=qk[0:64, KT + t, j0:j0 + kk],
                                rhs=qk[0:64, t, c0:c1],
                                start=True, stop=True)
                        for ci, (c0, c1) in enumerate(CH):
                            nc.tensor.matmul(
                                st1[0:kk, ci * PSTRIDE: ci * PSTRIDE + (c1 - c0)],
                                lhsT=qk[64:128, KT + t, j0:j0 + kk],
                                rhs=qk[64:128, t, c0:c1],
                                start=True, stop=True)
                        bias = mbias[0:kk, b * njt + jt: b * njt + jt + 1]
                        pt0 = ptpool.tile([128, W], bf16, tag="pt")
                        nc.scalar.activation(
                            cv(pt0)[0:kk], sv(st0)[0:kk],
                            mybir.ActivationFunctionType.Exp,
                            bias=bias, scale=SCALE)
                        pt1 = ptpool.tile([128, W], bf16, tag="pt")
                        nc.scalar.activation(
                            cv(pt1)[0:kk], sv(st1)[0:kk],
                            mybir.ActivationFunctionType.Exp,
                            bias=bias, scale=SCALE)
                        for ci, (c0, c1) in enumerate(CH):
                            nc.tensor.matmul(
                                av0[0:65, ci * PSTRIDE: ci * PSTRIDE + (c1 - c0)],
                                lhsT=v[0:kk, jt, h0 * 65:(h0 + 1) * 65],
                                rhs=pt0[0:kk, c0:c1],
                                start=(jt == 0), stop=(jt == njt - 1),
                                skip_group_check=True)
                        for ci, (c0, c1) in enumerate(CH):
                            nc.tensor.matmul(
                                av1[0:65, ci * PSTRIDE: ci * PSTRIDE + (c1 - c0)],
                                lhsT=v[0:kk, jt, h1 * 65:(h1 + 1) * 65],
                                rhs=pt1[0:kk, c0:c1],
                                start=(jt == 0), stop=(jt == njt - 1),
                                skip_group_check=True)

                    # evacuate av PSUM to SBUF fast: frees the shared PSUM
                    # slots for the next head pair; the normalize chain then
                    # runs off the PE critical path entirely in SBUF. h1
                    # first — its result needs an extra DMA hop into raw
                    # rows 64:128, so its chain gates the output projection.
                    ev = evpool.tile([65, 2, W], f32, tag="ev")
                    nc.vector.tensor_copy(
                        ev[:, 1, :].rearrange("p (c x) -> p c x", c=2),
                        sv(av1)[0:65])
                    # softmax denominator lives in ev row 64; stage it to
                    # partition 0 first (64->0 tensor_copy is the validated
                    # shift; custom DVE ops need partition-aligned reads).
                    den1 = rcpool.tile([1, W], f32, tag="den1")
                    nc.vector.tensor_copy(den1[:], ev[64:65, 1, :])
                    rc1 = rcpool.tile([1, W], f32, tag="rc1")
                    nc.vector.reciprocal_approx_fast(rc1[:], den1[:])
                    rcb1 = rcpool.tile([64, W], f32, tag="rcb1")
                    nc.gpsimd.partition_broadcast(rcb1[:], rc1[0:1, :])
                    # DVE tensor_tensor needs aligned partition bases:
                    # normalize h1 at rows 0:64; one consolidated DMA per
                    # batch item lifts all six halves to rows 64:128 (DMA
                    # dispatch is ~5us on HW, so batch them).
                    nc.vector.tensor_mul(stg[:, t, :], ev[0:64, 1, :], rcb1[:])

                    nc.vector.tensor_copy(
                        ev[:, 0, :].rearrange("p (c x) -> p c x", c=2),
                        sv(av0)[0:65])
                    den0 = rcpool.tile([1, W], f32, tag="den0")
                    nc.vector.tensor_copy(den0[:], ev[64:65, 0, :])
                    rc0 = rcpool.tile([1, W], f32, tag="rc0")
                    nc.vector.reciprocal_approx_fast(rc0[:], den0[:])
                    rcb0 = rcpool.tile([64, W], f32, tag="rcb0")
                    nc.gpsimd.partition_broadcast(rcb0[:], rc0[0:1, :])
                    nc.vector.tensor_mul(
                        raw[0:64, t, :], ev[0:64, 0, :], rcb0[:])

                nc.sync.dma_start(raw[64:128, :, :], stg[:])

                # ---- output projection (transposed out) ----
                # per-nt output DMAs so the store overlaps later nt tiles
                osb = opool.tile([128, KT, W], f32, tag="osb")
                for nt in range(KT):
                    ps = psA.tile([128, 1024], f32, tag="ps")
                    for kt in range(KT):
                        for ci, (c0, c1) in enumerate(CH):
                            nc.tensor.matmul(
                                ps[:, ci * PSTRIDE: ci * PSTRIDE + (c1 - c0)],
                                lhsT=pw[:, kt, nt * 128:(nt + 1) * 128],
                                rhs=raw[:, kt, c0:c1],
                                start=(kt == 0),
                                stop=(kt == KT - 1),
                            )
                    nc.vector.tensor_scalar_add(
                        cv(osb[:, nt, :]), sv(ps), pb[:, nt:nt + 1])
                nc.sync.dma_start(outTr[:, :, b * lp: b * lp + W], osb[:])

    nc.compile()
    return nc


def _host_prep(x, mb, qkv_w, qkv_b, proj_w, proj_b, alpha, bpc=BPC, lp=LP,
               ncores=NCORES, l=L):
    """Shard + lay out inputs. Returns in_maps (one dict per core)."""
    import ml_dtypes

    bf16 = ml_dtypes.bfloat16
    njt = lp // 128
    x = np.asarray(x, np.float32)
    mb = np.asarray(mb, np.float32)
    qkv_w = np.asarray(qkv_w, np.float32)
    qkv_b = np.asarray(qkv_b, np.float32)
    proj_w = np.asarray(proj_w, np.float32)
    proj_b = np.asarray(proj_b, np.float32)
    alpha = np.asarray(alpha, np.float32)

    wqkvT = np.ascontiguousarray(qkv_w.T).astype(bf16)         # [768, 2304]
    pwT = np.ascontiguousarray(proj_w.T).astype(bf16)          # [768, 768]
    # q bias only (k bias is softmax-invariant and dropped)
    bqkT = np.ascontiguousarray(qkv_b[:D].reshape(KT, 128).T)
    # v bias folded through the projection into the proj bias
    pb2 = proj_b + proj_w @ qkv_b[2 * D:]
    pbT = np.ascontiguousarray(pb2.reshape(KT, 128).T).astype(np.float32)

    # key bias: alpha * mb (with cls zero) + pad mask, same layout as scores
    mask1 = np.zeros(lp, np.float32)
    mask1[l:] = NEG

    in_maps = []
    for c in range(ncores):
        xb = x[c * bpc:(c + 1) * bpc]                          # [bpc, L, D]
        xp = np.zeros((bpc, lp, D), np.float32)
        xp[:, :l, :] = xb
        xTc = np.ascontiguousarray(xp.reshape(bpc * lp, D).T).astype(bf16)

        mbb = mb[c * bpc:(c + 1) * bpc]                        # [bpc, L-1]
        mbp = np.tile(mask1, (bpc, 1))                         # [bpc, lp]
        mbp[:, 1:l] += alpha[0] * mbb
        mbTc = np.ascontiguousarray(
            mbp.reshape(bpc * njt, 128).T)                     # [128, bpc*njt]

        in_maps.append({
            "xT": xTc, "wqkvT": wqkvT, "pwT": pwT, "bqkT": bqkT,
            "pbT": pbT, "mbT": mbTc,
        })
    return in_maps


def _host_gather(outs, bpc=BPC, lp=LP, l=L):
    """outs: list of {'outT': [768, tok]} per core -> [B, L, D] fp32."""
    parts = []
    for o in outs:
        t = np.asarray(o["outT"]).T.reshape(bpc, lp, D)[:, :l, :]
        parts.append(t)
    return np.ascontiguousarray(np.concatenate(parts, 0)).astype(np.float32)


def kernel(x, mb, qkv_w, qkv_b, proj_w, proj_b, alpha):
    from concourse.bass_utils import run_bass_kernel_spmd

    if "nc" not in _CACHE:
        _CACHE["nc"] = _build()
    nc = _CACHE["nc"]
    in_maps = _host_prep(x, mb, qkv_w, qkv_b, proj_w, proj_b, alpha)
    res = run_bass_kernel_spmd(nc, in_maps, core_ids=list(range(NCORES)))
    return _host_gather(res.results)
